# revision 1
# baseline (speedup 1.0000x reference)
"""Trainium2 Bass kernel for multi-graph SpMM propagation (GNN message passing).

Computation (per graph): f0 = feat; f_{l+1} = segsum(vals * f_l[cols], rows);
kernel outputs raw f1, f2 shards (fp16). Host applies l2norm + layer average.

Strategy: 1D row-partition each graph across 8 cores; one identical SPMD
program for all cores (per-(block,bucket) tile counts padded to the max over
cores). Edge tiles of 128; per tile:
  - dma_gather (256B fp32 rows) fetches feat[col]; int16 indices are relative
    to a 32768-row column bucket window; calls cover <=8 tiles (SWDGE ring cap)
  - ScalarE casts gathered fp32 -> fp16 into a per-segment buffer
  - VectorE dual-op tensor_scalar builds a val-weighted one-hot vs iota const
  - PE matmul (fp16, FWL) accumulates onehot.T @ gathered into the block PSUM
Blocks are grouped into segments (16 blocks); gathers are bucket-major within
a segment, matmuls block-major. Layer boundary: fp16 AllGather per graph.
"""

import numpy as np

import concourse.bacc as bacc
import concourse.bass as bass
import concourse.mybir as mybir
import concourse.tile as tile
from concourse.bass_utils import run_bass_kernel_spmd

NCORES = 8
P = 128
D = 64
BUCKET = 65536       # column window per gather call (int16 idx, signed range)
HALF = 32768
CALL_TILES = 8       # tiles per dma_gather call (1024 idx = SWDGE ring cap)
SEG_BLOCKS = 16      # blocks per segment

U, NI, NB = 100000, 50000, 20000


def _preprocess_graph(rows, cols, vals, n_total):
    rows = np.asarray(rows, dtype=np.int64)
    cols = np.asarray(cols, dtype=np.int64)
    vals = np.asarray(vals, dtype=np.float32)

    S = ((n_total + NCORES * P - 1) // (NCORES * P)) * P
    nblocks = S // P
    nbuckets = (NCORES * S + BUCKET - 1) // BUCKET
    nsegs = (nblocks + SEG_BLOCKS - 1) // SEG_BLOCKS

    core = np.minimum(rows // S, NCORES - 1)
    blk = (rows - core * S) // P
    bkt = cols // BUCKET
    seg = blk // SEG_BLOCKS

    # per-bucket gather base: mid-window so int16 covers 65536 rows; clamped
    # near the table end so every tile keeps a non-negative (untrimmable) idx
    bases = []
    for B in range(nbuckets):
        size = n_total - B * BUCKET
        if size <= 0:
            bases.append(min(B * BUCKET, n_total - 1))
        elif size <= HALF:
            bases.append(B * BUCKET)
        elif size <= BUCKET:
            bases.append(n_total - HALF)
        else:
            bases.append(B * BUCKET + HALF)
    bases = np.array(bases, np.int64)

    # group id in slot order: (seg, bucket, block); per-core counts
    grp = (seg * nbuckets + bkt) * nblocks + blk
    ngrp = nsegs * nbuckets * nblocks
    counts = np.zeros((NCORES, ngrp), np.int64)
    np.add.at(counts, (core, grp), 1)

    tiles_per_grp = -(-counts.max(axis=0) // P)  # [ngrp]
    # Every block must produce >=1 tile so its PSUM/output gets written.
    tpg3 = tiles_per_grp.reshape(nsegs, nbuckets, nblocks)
    blk_tiles = tpg3.sum(axis=(0, 1))
    for b in np.where(blk_tiles == 0)[0]:
        tpg3[b // SEG_BLOCKS, 0, b] = 1
    tiles_per_grp = tpg3.reshape(-1)

    grp_tile_start = np.concatenate([[0], np.cumsum(tiles_per_grp)])
    T_tot = int(grp_tile_start[-1])

    # place edges into slots (rank within each (grp, core) run)
    key = grp * NCORES + core
    order = np.argsort(key, kind="stable")
    key_s = key[order]
    run_start = np.concatenate([[0], np.cumsum(counts.reshape(NCORES, ngrp).T.reshape(-1))])
    rank = np.arange(len(rows)) - run_start[key_s]
    slot = grp_tile_start[grp[order]] * P + rank
    c_s = core[order]

    col_rel = np.zeros((NCORES, T_tot * P), np.int16)
    rl_slots = np.zeros((NCORES, T_tot * P), np.float32)
    val_slots = np.zeros((NCORES, T_tot * P), np.float32)
    rel = cols[order] - bases[bkt[order]]
    assert rel.min() >= -HALF and rel.max() < HALF
    col_rel[c_s, slot] = rel.astype(np.int16)
    rl_slots[c_s, slot] = (rows[order] - c_s * S - blk[order] * P).astype(np.float32)
    val_slots[c_s, slot] = vals[order].astype(np.float32)

    # sort each tile's lanes by idx ascending (trailing idx of every gather
    # call must be >= 0: the Q7 kernel trims trailing negatives)
    cr = col_rel.reshape(NCORES, T_tot, P)
    rl2 = rl_slots.reshape(NCORES, T_tot, P)
    vl2 = val_slots.reshape(NCORES, T_tot, P)
    perm = np.argsort(cr, axis=2, kind="stable")
    cr[:] = np.take_along_axis(cr, perm, axis=2)
    rl2[:] = np.take_along_axis(rl2, perm, axis=2)
    vl2[:] = np.take_along_axis(vl2, perm, axis=2)
    assert (cr[:, :, P - 1] >= 0).all()

    # schedules
    segs = []
    idx_chunks = []  # per-call [NCORES, 16, nt*8] int16
    call_col_off = 0
    col_rel_t = col_rel.reshape(NCORES, T_tot, P)
    for s in range(nsegs):
        b_lo = s * SEG_BLOCKS
        b_hi = min(nblocks, b_lo + SEG_BLOCKS)
        seg_t0 = int(grp_tile_start[(s * nbuckets + 0) * nblocks + b_lo])
        calls = []
        for B in range(nbuckets):
            g0 = (s * nbuckets + B) * nblocks
            t0 = int(grp_tile_start[g0 + b_lo])
            t1 = int(grp_tile_start[g0 + b_hi - 1] + tiles_per_grp[g0 + b_hi - 1])
            t = t0
            while t < t1:
                nt = min(CALL_TILES, t1 - t)
                calls.append(dict(bucket=B, t0=t, nt=nt, col_off=call_col_off))
                chunk = col_rel_t[:, t : t + nt, :].reshape(NCORES, nt * P)
                wrapped = np.zeros((NCORES, 16, nt * 8), np.int16)
                i = np.arange(nt * P)
                wrapped[:, i % 16, i // 16] = chunk
                idx_chunks.append(wrapped)
                call_col_off += nt * 8
                t += nt
        seg_t1 = t if calls else seg_t0
        blocks = []
        for b in range(b_lo, b_hi):
            tl = []
            for B in range(nbuckets):
                g = (s * nbuckets + B) * nblocks + b
                tl.extend(range(int(grp_tile_start[g]),
                                int(grp_tile_start[g]) + int(tiles_per_grp[g])))
            blocks.append((b, tl))
        segs.append(dict(seg_t0=seg_t0, seg_t1=seg_t1, calls=calls, blocks=blocks))

    idx_all = np.concatenate(idx_chunks, axis=2)  # [NCORES, 16, cols]
    idx_all = np.tile(idx_all, (1, 8, 1))         # replicate to 128 partitions

    def to_tiles(a):
        return np.ascontiguousarray(a.reshape(NCORES, T_tot, P).transpose(0, 2, 1))

    return dict(
        n_total=n_total, S=S, nblocks=nblocks, nbuckets=nbuckets, bases=bases,
        T_tot=T_tot, segs=segs, idx_ncols=idx_all.shape[2],
        idx=np.ascontiguousarray(idx_all),
        rl=to_tiles(rl_slots), val=to_tiles(val_slots),
    )


def _build_program(metas, graph_order):
    f16 = mybir.dt.float16
    f32 = mybir.dt.float32
    i16 = mybir.dt.int16

    nc = bacc.Bacc(
        "TRN2", target_bir_lowering=False, debug=False,
        enable_asserts=False, num_devices=NCORES,
    )

    tabs, idxs, rls, vls, f1s, f2s = {}, {}, {}, {}, {}, {}
    for g in graph_order:
        m = metas[g]
        tabs[g] = nc.dram_tensor(f"table_{g}", [m["n_total"], D], f32, kind="ExternalInput")
        idxs[g] = nc.dram_tensor(f"idx_{g}", [P, m["idx_ncols"]], i16, kind="ExternalInput")
        rls[g] = nc.dram_tensor(f"rl_{g}", [P, m["T_tot"]], f32, kind="ExternalInput")
        vls[g] = nc.dram_tensor(f"val_{g}", [P, m["T_tot"]], f32, kind="ExternalInput")
        f1s[g] = nc.dram_tensor(f"f1_{g}", [m["S"], D], f16, kind="ExternalOutput")
        f2s[g] = nc.dram_tensor(f"f2_{g}", [m["S"], D], f16, kind="ExternalOutput")

    seg_tiles_max = max(
        max(s["seg_t1"] - s["seg_t0"] for s in metas[g]["segs"]) for g in graph_order
    )
    seg_cols_max = max(
        max(max(sum(c["nt"] for c in s["calls"]) * 8, 1) for s in metas[g]["segs"])
        for g in graph_order
    )

    with tile.TileContext(nc) as tc:
        with (
            tc.tile_pool(name="const", bufs=1) as cpool,
            tc.tile_pool(name="meta", bufs=1) as mpool,
            tc.tile_pool(name="idxp", bufs=3) as ipool,
            tc.tile_pool(name="exp16", bufs=2) as e16pool,
            tc.tile_pool(name="exp32", bufs=2) as e32pool,
            tc.tile_pool(name="segp", bufs=2) as spool,
            tc.tile_pool(name="tmpp", bufs=6) as tpool,
            tc.tile_pool(name="oneh", bufs=16) as opool,
            tc.tile_pool(name="fout", bufs=8) as fpool,
            tc.tile_pool(name="psum", bufs=6, space="PSUM") as ppool,
            tc.tile_pool(name="dram", bufs=1, space="DRAM") as dpool,
        ):
            iota_t = cpool.tile([P, P], i16, name="iota_t")
            nc.gpsimd.iota(iota_t[:], pattern=[[1, P]], base=0, channel_multiplier=0)

            rl_sb, val_sb, ag_in, table2, table2f32 = {}, {}, {}, {}, {}
            for g in graph_order:
                m = metas[g]
                rl_sb[g] = mpool.tile([P, m["T_tot"]], f32, tag=f"rl_{g}", name=f"rl_sb_{g}")
                val_sb[g] = mpool.tile([P, m["T_tot"]], f32, tag=f"val_{g}", name=f"val_sb_{g}")
                nc.sync.dma_start(rl_sb[g][:], rls[g][:])
                nc.sync.dma_start(val_sb[g][:], vls[g][:])
                ag_in[g] = dpool.tile([m["S"], D], f16, tag=f"agin_{g}", name=f"ag_in_{g}")
                table2[g] = dpool.tile([m["S"] * NCORES, D], f16, tag=f"tab2_{g}", name=f"table2_{g}")
                table2f32[g] = dpool.tile([m["S"] * NCORES, D], f32, tag=f"tab2f_{g}", name=f"table2f32_{g}")

            def emit_layer(g, table_ap_fn, dst_drams):
                m = metas[g]
                for s in m["segs"]:
                    seg_t0 = s["seg_t0"]
                    segbuf = None
                    if s["calls"]:
                        ncols = sum(c["nt"] for c in s["calls"]) * 8
                        idxchunk = ipool.tile([P, seg_cols_max], i16, tag="idxc", name="idxchunk")
                        c0 = s["calls"][0]["col_off"]
                        nc.sync.dma_start(idxchunk[:, :ncols], idxs[g][:, c0 : c0 + ncols])
                        segbuf = spool.tile([P, seg_tiles_max, D], f16, tag="segbuf", name="segbuf")
                        for call in s["calls"]:
                            nt = call["nt"]
                            tmp = tpool.tile([P, CALL_TILES, D], f32, tag="gtmp", name="gtmp")
                            nc.gpsimd.dma_gather(
                                out_ap=tmp[:, :nt, :],
                                in_ap=table_ap_fn(call["bucket"]),
                                idxs_ap=idxchunk[:, call["col_off"] - c0 : call["col_off"] - c0 + nt * 8],
                                num_idxs=nt * P,
                                num_idxs_reg=nt * P,
                                elem_size=D,
                            )
                            so = call["t0"] - seg_t0
                            nc.scalar.activation(
                                segbuf[:, so : so + nt, :], tmp[:, :nt, :],
                                mybir.ActivationFunctionType.Copy,
                            )
                    for b, tl in s["blocks"]:
                        psum = ppool.tile([P, D], f32, tag="psum", name="psum")
                        for i, t in enumerate(tl):
                            onehot = opool.tile([P, P], f16, tag="oneh", name="onehot")
                            nc.vector.tensor_scalar(
                                onehot[:], iota_t[:],
                                rl_sb[g][:, t : t + 1], val_sb[g][:, t : t + 1],
                                mybir.AluOpType.is_equal, mybir.AluOpType.mult,
                            )
                            nc.tensor.matmul(
                                out=psum[:], lhsT=onehot[:],
                                rhs=segbuf[:, t - seg_t0, :],
                                start=(i == 0), stop=(i == len(tl) - 1),
                            )
                        f16t = fpool.tile([P, D], f16, tag="fout", name="f16t")
                        nc.scalar.activation(f16t[:], psum[:], mybir.ActivationFunctionType.Copy)
                        for dd in dst_drams:
                            nc.sync.dma_start(dd[b * P : (b + 1) * P, :], f16t[:])

            RW = 32  # table2 fp16->fp32 expansion: rows per partition per chunk

            def emit_expand(g):
                m = metas[g]
                nrows = m["S"] * NCORES
                r0 = 0
                while r0 < nrows:
                    rw = min(RW, (nrows - r0) // P)
                    ch = rw * D
                    sb16 = e16pool.tile([P, RW * D], f16, tag="e16", name="sb16")
                    sb32 = e32pool.tile([P, RW * D], f32, tag="e32", name="sb32")
                    src16 = table2[g][r0 : r0 + P * rw, :].rearrange(
                        "(p r) d -> p (r d)", p=P)
                    dst32 = table2f32[g][r0 : r0 + P * rw, :].rearrange(
                        "(p r) d -> p (r d)", p=P)
                    nc.sync.dma_start(sb16[:, :ch], src16)
                    nc.vector.tensor_copy(sb32[:, :ch], sb16[:, :ch])
                    nc.sync.dma_start(dst32, sb32[:, :ch])
                    r0 += P * rw

            rg = [list(range(NCORES))]
            for g in graph_order:
                emit_layer(g, lambda B, g=g: tabs[g][int(metas[g]['bases'][B]) :, :], [ag_in[g], f1s[g]])
                nc.gpsimd.collective_compute(
                    "AllGather", mybir.AluOpType.bypass, replica_groups=rg,
                    ins=[ag_in[g][:]], outs=[table2[g][:]],
                )
            for g in graph_order:
                emit_expand(g)
            for g in graph_order:
                emit_layer(g, lambda B, g=g: table2f32[g][int(metas[g]['bases'][B]) :, :], [f2s[g]])

    nc.compile()
    return nc


def _l2norm_rows(x):
    x = x.astype(np.float32)
    n = np.sqrt(np.sum(x * x, axis=1, keepdims=True))
    return x / np.maximum(n, 1e-12)


def _make_in_maps(graphs, graph_order, metas):
    in_maps = []
    for k in range(NCORES):
        im = {}
        for g in graph_order:
            feat = graphs[g][0]
            m = metas[g]
            im[f"table_{g}"] = np.ascontiguousarray(feat.astype(np.float32))
            im[f"idx_{g}"] = np.ascontiguousarray(m["idx"][k])
            im[f"rl_{g}"] = np.ascontiguousarray(m["rl"][k])
            im[f"val_{g}"] = np.ascontiguousarray(m["val"][k])
        in_maps.append(im)
    return in_maps


def _run(graphs, graph_order, run_fn=None):
    metas = {g: _preprocess_graph(graphs[g][1], graphs[g][2], graphs[g][3],
                                  graphs[g][0].shape[0]) for g in graph_order}
    nc = _build_program(metas, graph_order)
    in_maps = _make_in_maps(graphs, graph_order, metas)

    if run_fn is None:
        results = run_bass_kernel_spmd(nc, in_maps, core_ids=list(range(NCORES))).results
    else:
        results = run_fn(nc, in_maps)

    out = {}
    for g in graph_order:
        m = metas[g]
        n = m["n_total"]
        f1 = np.concatenate([results[k][f"f1_{g}"] for k in range(NCORES)], axis=0)
        f2 = np.concatenate([results[k][f"f2_{g}"] for k in range(NCORES)], axis=0)
        out[g] = (f1[:n].astype(np.float32), f2[:n].astype(np.float32))
    return out


def kernel(
    users_feature, items_feature, bundles_feature,
    ui_vals, bi_vals, ub_vals,
    ui_rows, ui_cols, bi_rows, bi_cols, ub_rows, ub_cols,
):
    users_feature = np.asarray(users_feature, dtype=np.float32)
    items_feature = np.asarray(items_feature, dtype=np.float32)
    bundles_feature = np.asarray(bundles_feature, dtype=np.float32)

    feats = {
        "ui": np.concatenate([users_feature, items_feature], axis=0),
        "bi": np.concatenate([bundles_feature, items_feature], axis=0),
        "ub": np.concatenate([users_feature, bundles_feature], axis=0),
    }
    graphs = {
        "ui": (feats["ui"], ui_rows, ui_cols, ui_vals),
        "bi": (feats["bi"], bi_rows, bi_cols, bi_vals),
        "ub": (feats["ub"], ub_rows, ub_cols, ub_vals),
    }
    graph_order = ["bi", "ub", "ui"]

    fs = _run(graphs, graph_order)

    agg = {}
    for g in graph_order:
        f1, f2 = fs[g]
        agg[g] = (feats[g] + _l2norm_rows(f1) + _l2norm_rows(f2)) / 3.0

    return np.concatenate(
        [
            agg["ui"][:U],
            agg["ub"][:U],
            agg["bi"][:NB],
            agg["ub"][U : U + NB],
            agg["ui"][U : U + NI],
            agg["bi"][NB : NB + NI],
        ],
        axis=0,
    ).astype(np.float32)



# revision 3
# speedup vs baseline: 2.0841x; 2.0841x over previous
"""Trainium2 Bass kernel for multi-graph SpMM propagation (GNN message passing).

Computation (per graph): f0 = feat; f_{l+1} = segsum(vals * f_l[cols], rows);
kernel outputs raw f1, f2 shards (fp16). Host applies l2norm + layer average.

Strategy: 1D row-partition each graph across 8 cores; one identical SPMD
program for all cores (per-(block,bucket) tile counts padded to the max over
cores). Edge tiles of 128; per tile:
  - dma_gather (256B fp32 rows) fetches feat[col]; int16 indices are relative
    to a 32768-row column bucket window; calls cover <=8 tiles (SWDGE ring cap)
  - ScalarE casts gathered fp32 -> fp16 into a per-segment buffer
  - VectorE dual-op tensor_scalar builds a val-weighted one-hot vs iota const
  - PE matmul (fp16, FWL) accumulates onehot.T @ gathered into the block PSUM
Blocks are grouped into segments (16 blocks); gathers are bucket-major within
a segment, matmuls block-major. Layer boundary: fp16 AllGather per graph.
"""

import numpy as np

import concourse.bacc as bacc
import concourse.bass as bass
import concourse.mybir as mybir
import concourse.tile as tile
from concourse.bass_utils import run_bass_kernel_spmd

NCORES = 8
P = 128
D = 64
BUCKET = 65536       # column window per gather call (int16 idx, signed range)
HALF = 32768
CALL_TILES = 8       # tiles per dma_gather call (1024 idx = SWDGE ring cap)
SEG_BLOCKS = 16      # blocks per segment

U, NI, NB = 100000, 50000, 20000


def _preprocess_graph(rows, cols, vals, n_total):
    rows = np.asarray(rows, dtype=np.int64)
    cols = np.asarray(cols, dtype=np.int64)
    vals = np.asarray(vals, dtype=np.float32)

    S = ((n_total + NCORES * P - 1) // (NCORES * P)) * P
    nblocks = S // P
    nbuckets = (NCORES * S + BUCKET - 1) // BUCKET
    nsegs = (nblocks + SEG_BLOCKS - 1) // SEG_BLOCKS

    core = np.minimum(rows // S, NCORES - 1)
    blk = (rows - core * S) // P
    bkt = cols // BUCKET
    seg = blk // SEG_BLOCKS

    # per-bucket gather base: mid-window so int16 covers 65536 rows; clamped
    # near the table end so every tile keeps a non-negative (untrimmable) idx
    bases = []
    for B in range(nbuckets):
        size = n_total - B * BUCKET
        if size <= 0:
            bases.append(min(B * BUCKET, n_total - 1))
        elif size <= HALF:
            bases.append(B * BUCKET)
        elif size <= BUCKET:
            bases.append(n_total - HALF)
        else:
            bases.append(B * BUCKET + HALF)
    bases = np.array(bases, np.int64)

    # group id in slot order: (seg, bucket, block); per-core counts
    grp = (seg * nbuckets + bkt) * nblocks + blk
    ngrp = nsegs * nbuckets * nblocks
    counts = np.zeros((NCORES, ngrp), np.int64)
    np.add.at(counts, (core, grp), 1)

    tiles_per_grp = -(-counts.max(axis=0) // P)  # [ngrp]
    # Every block must produce >=1 tile so its PSUM/output gets written.
    tpg3 = tiles_per_grp.reshape(nsegs, nbuckets, nblocks)
    blk_tiles = tpg3.sum(axis=(0, 1))
    for b in np.where(blk_tiles == 0)[0]:
        tpg3[b // SEG_BLOCKS, 0, b] = 1
    tiles_per_grp = tpg3.reshape(-1)

    grp_tile_start = np.concatenate([[0], np.cumsum(tiles_per_grp)])
    T_tot = int(grp_tile_start[-1])

    # place edges into slots (rank within each (grp, core) run)
    key = grp * NCORES + core
    order = np.argsort(key, kind="stable")
    key_s = key[order]
    run_start = np.concatenate([[0], np.cumsum(counts.reshape(NCORES, ngrp).T.reshape(-1))])
    rank = np.arange(len(rows)) - run_start[key_s]
    slot = grp_tile_start[grp[order]] * P + rank
    c_s = core[order]

    col_rel = np.zeros((NCORES, T_tot * P), np.int16)
    rl_slots = np.zeros((NCORES, T_tot * P), np.float32)
    val_slots = np.zeros((NCORES, T_tot * P), np.float32)
    rel = cols[order] - bases[bkt[order]]
    assert rel.min() >= -HALF and rel.max() < HALF
    col_rel[c_s, slot] = rel.astype(np.int16)
    rl_slots[c_s, slot] = (rows[order] - c_s * S - blk[order] * P).astype(np.float32)
    val_slots[c_s, slot] = vals[order].astype(np.float32)

    # sort each tile's lanes by idx ascending (trailing idx of every gather
    # call must be >= 0: the Q7 kernel trims trailing negatives)
    cr = col_rel.reshape(NCORES, T_tot, P)
    rl2 = rl_slots.reshape(NCORES, T_tot, P)
    vl2 = val_slots.reshape(NCORES, T_tot, P)
    perm = np.argsort(cr, axis=2, kind="stable")
    cr[:] = np.take_along_axis(cr, perm, axis=2)
    rl2[:] = np.take_along_axis(rl2, perm, axis=2)
    vl2[:] = np.take_along_axis(vl2, perm, axis=2)
    assert (cr[:, :, P - 1] >= 0).all()

    # schedules
    segs = []
    idx_chunks = []  # per-call [NCORES, 16, nt*8] int16
    call_col_off = 0
    col_rel_t = col_rel.reshape(NCORES, T_tot, P)
    for s in range(nsegs):
        b_lo = s * SEG_BLOCKS
        b_hi = min(nblocks, b_lo + SEG_BLOCKS)
        seg_t0 = int(grp_tile_start[(s * nbuckets + 0) * nblocks + b_lo])
        calls = []
        for B in range(nbuckets):
            g0 = (s * nbuckets + B) * nblocks
            t0 = int(grp_tile_start[g0 + b_lo])
            t1 = int(grp_tile_start[g0 + b_hi - 1] + tiles_per_grp[g0 + b_hi - 1])
            t = t0
            while t < t1:
                nt = min(CALL_TILES, t1 - t)
                calls.append(dict(bucket=B, t0=t, nt=nt, col_off=call_col_off))
                chunk = col_rel_t[:, t : t + nt, :].reshape(NCORES, nt * P)
                wrapped = np.zeros((NCORES, 16, nt * 8), np.int16)
                i = np.arange(nt * P)
                wrapped[:, i % 16, i // 16] = chunk
                idx_chunks.append(wrapped)
                call_col_off += nt * 8
                t += nt
        seg_t1 = t if calls else seg_t0
        blocks = []
        for b in range(b_lo, b_hi):
            tl = []
            for B in range(nbuckets):
                g = (s * nbuckets + B) * nblocks + b
                tl.extend(range(int(grp_tile_start[g]),
                                int(grp_tile_start[g]) + int(tiles_per_grp[g])))
            blocks.append((b, tl))
        segs.append(dict(seg_t0=seg_t0, seg_t1=seg_t1, calls=calls, blocks=blocks))

    idx_all = np.concatenate(idx_chunks, axis=2)  # [NCORES, 16, cols]
    idx_all = np.tile(idx_all, (1, 8, 1))         # replicate to 128 partitions

    def to_tiles(a):
        return np.ascontiguousarray(a.reshape(NCORES, T_tot, P).transpose(0, 2, 1))

    return dict(
        n_total=n_total, S=S, nblocks=nblocks, nbuckets=nbuckets, bases=bases,
        T_tot=T_tot, segs=segs, idx_ncols=idx_all.shape[2],
        idx=np.ascontiguousarray(idx_all),
        rl=to_tiles(rl_slots), val=to_tiles(val_slots),
    )


def _build_program(metas, graph_order, repeat=1):
    f16 = mybir.dt.float16
    f32 = mybir.dt.float32
    i16 = mybir.dt.int16

    nc = bacc.Bacc(
        "TRN2", target_bir_lowering=False, debug=False,
        enable_asserts=False, num_devices=NCORES,
    )

    tabs, idxs, rls, vls, f1s, f2s = {}, {}, {}, {}, {}, {}
    for g in graph_order:
        m = metas[g]
        tabs[g] = nc.dram_tensor(f"table_{g}", [m["n_total"], D], f32, kind="ExternalInput")
        idxs[g] = nc.dram_tensor(f"idx_{g}", [P, m["idx_ncols"]], i16, kind="ExternalInput")
        rls[g] = nc.dram_tensor(f"rl_{g}", [P, m["T_tot"]], f32, kind="ExternalInput")
        vls[g] = nc.dram_tensor(f"val_{g}", [P, m["T_tot"]], f32, kind="ExternalInput")
        f1s[g] = nc.dram_tensor(f"f1_{g}", [m["S"], D], f16, kind="ExternalOutput")
        f2s[g] = nc.dram_tensor(f"f2_{g}", [m["S"], D], f16, kind="ExternalOutput")

    seg_tiles_max = max(
        max(s["seg_t1"] - s["seg_t0"] for s in metas[g]["segs"]) for g in graph_order
    )
    seg_cols_max = max(
        max(max(sum(c["nt"] for c in s["calls"]) * 8, 1) for s in metas[g]["segs"])
        for g in graph_order
    )

    with tile.TileContext(nc) as tc:
        with (
            tc.tile_pool(name="const", bufs=1) as cpool,
            tc.tile_pool(name="meta", bufs=1) as mpool,
            tc.tile_pool(name="idxp", bufs=3) as ipool,
            tc.tile_pool(name="exp16", bufs=2) as e16pool,
            tc.tile_pool(name="exp32", bufs=2) as e32pool,
            tc.tile_pool(name="segp", bufs=2) as spool,
            tc.tile_pool(name="tmpp", bufs=6) as tpool,
            tc.tile_pool(name="oneh", bufs=16) as opool,
            tc.tile_pool(name="fout", bufs=8) as fpool,
            tc.tile_pool(name="psum", bufs=6, space="PSUM") as ppool,
            tc.tile_pool(name="dram", bufs=1, space="DRAM") as dpool,
        ):
            iota_t = cpool.tile([P, P], i16, name="iota_t")
            nc.gpsimd.iota(iota_t[:], pattern=[[1, P]], base=0, channel_multiplier=0)

            rl_sb, val_sb, ag_in, table2, table2f32 = {}, {}, {}, {}, {}
            for g in graph_order:
                m = metas[g]
                rl_sb[g] = mpool.tile([P, m["T_tot"]], f32, tag=f"rl_{g}", name=f"rl_sb_{g}")
                val_sb[g] = mpool.tile([P, m["T_tot"]], f32, tag=f"val_{g}", name=f"val_sb_{g}")
                nc.sync.dma_start(rl_sb[g][:], rls[g][:])
                nc.sync.dma_start(val_sb[g][:], vls[g][:])
                ag_in[g] = dpool.tile([m["S"], D], f16, tag=f"agin_{g}", name=f"ag_in_{g}")
                table2[g] = dpool.tile([m["S"] * NCORES, D], f16, tag=f"tab2_{g}", name=f"table2_{g}")
                table2f32[g] = dpool.tile([m["S"] * NCORES, D], f32, tag=f"tab2f_{g}", name=f"table2f32_{g}")

            def emit_layer(g, table_ap_fn, dst_drams):
                m = metas[g]
                for s in m["segs"]:
                    seg_t0 = s["seg_t0"]
                    segbuf = None
                    if s["calls"]:
                        ncols = sum(c["nt"] for c in s["calls"]) * 8
                        idxchunk = ipool.tile([P, seg_cols_max], i16, tag="idxc", name="idxchunk")
                        c0 = s["calls"][0]["col_off"]
                        nc.sync.dma_start(idxchunk[:, :ncols], idxs[g][:, c0 : c0 + ncols])
                        segbuf = spool.tile([P, seg_tiles_max, D], f16, tag="segbuf", name="segbuf")
                        for call in s["calls"]:
                            nt = call["nt"]
                            tmp = tpool.tile([P, CALL_TILES, D], f32, tag="gtmp", name="gtmp")
                            nc.gpsimd.dma_gather(
                                out_ap=tmp[:, :nt, :],
                                in_ap=table_ap_fn(call["bucket"]),
                                idxs_ap=idxchunk[:, call["col_off"] - c0 : call["col_off"] - c0 + nt * 8],
                                num_idxs=nt * P,
                                num_idxs_reg=nt * P,
                                elem_size=D,
                            )
                            so = call["t0"] - seg_t0
                            nc.scalar.activation(
                                segbuf[:, so : so + nt, :], tmp[:, :nt, :],
                                mybir.ActivationFunctionType.Copy,
                            )
                    for b, tl in s["blocks"]:
                        psum = ppool.tile([P, D], f32, tag="psum", name="psum")
                        for i, t in enumerate(tl):
                            onehot = opool.tile([P, P], f16, tag="oneh", name="onehot")
                            nc.vector.tensor_scalar(
                                onehot[:], iota_t[:],
                                rl_sb[g][:, t : t + 1], val_sb[g][:, t : t + 1],
                                mybir.AluOpType.is_equal, mybir.AluOpType.mult,
                            )
                            nc.tensor.matmul(
                                out=psum[:], lhsT=onehot[:],
                                rhs=segbuf[:, t - seg_t0, :],
                                start=(i == 0), stop=(i == len(tl) - 1),
                            )
                        f16t = fpool.tile([P, D], f16, tag="fout", name="f16t")
                        nc.scalar.activation(f16t[:], psum[:], mybir.ActivationFunctionType.Copy)
                        for dd in dst_drams:
                            nc.sync.dma_start(dd[b * P : (b + 1) * P, :], f16t[:])

            RW = 32  # table2 fp16->fp32 expansion: rows per partition per chunk

            def emit_expand(g):
                m = metas[g]
                nrows = m["S"] * NCORES
                r0 = 0
                while r0 < nrows:
                    rw = min(RW, (nrows - r0) // P)
                    ch = rw * D
                    sb16 = e16pool.tile([P, RW * D], f16, tag="e16", name="sb16")
                    sb32 = e32pool.tile([P, RW * D], f32, tag="e32", name="sb32")
                    src16 = table2[g][r0 : r0 + P * rw, :].rearrange(
                        "(p r) d -> p (r d)", p=P)
                    dst32 = table2f32[g][r0 : r0 + P * rw, :].rearrange(
                        "(p r) d -> p (r d)", p=P)
                    nc.sync.dma_start(sb16[:, :ch], src16)
                    nc.vector.tensor_copy(sb32[:, :ch], sb16[:, :ch])
                    nc.sync.dma_start(dst32, sb32[:, :ch])
                    r0 += P * rw

            rg = [list(range(NCORES))]
            for _rep in range(repeat):
                for g in graph_order:
                    emit_layer(g, lambda B, g=g: tabs[g][int(metas[g]['bases'][B]) :, :], [ag_in[g], f1s[g]])
                    nc.gpsimd.collective_compute(
                        "AllGather", mybir.AluOpType.bypass, replica_groups=rg,
                        ins=[ag_in[g][:]], outs=[table2[g][:]],
                    )
                for g in graph_order:
                    emit_expand(g)
                for g in graph_order:
                    emit_layer(g, lambda B, g=g: table2f32[g][int(metas[g]['bases'][B]) :, :], [f2s[g]])

    nc.compile()
    return nc


def _l2norm_rows(x):
    x = x.astype(np.float32)
    n = np.sqrt(np.sum(x * x, axis=1, keepdims=True))
    return x / np.maximum(n, 1e-12)


def _make_in_maps(graphs, graph_order, metas):
    in_maps = []
    for k in range(NCORES):
        im = {}
        for g in graph_order:
            feat = graphs[g][0]
            m = metas[g]
            im[f"table_{g}"] = np.ascontiguousarray(feat.astype(np.float32))
            im[f"idx_{g}"] = np.ascontiguousarray(m["idx"][k])
            im[f"rl_{g}"] = np.ascontiguousarray(m["rl"][k])
            im[f"val_{g}"] = np.ascontiguousarray(m["val"][k])
        in_maps.append(im)
    return in_maps


def _run(graphs, graph_order, run_fn=None):
    metas = {g: _preprocess_graph(graphs[g][1], graphs[g][2], graphs[g][3],
                                  graphs[g][0].shape[0]) for g in graph_order}
    nc = _build_program(metas, graph_order)
    in_maps = _make_in_maps(graphs, graph_order, metas)

    if run_fn is None:
        results = run_bass_kernel_spmd(nc, in_maps, core_ids=list(range(NCORES))).results
    else:
        results = run_fn(nc, in_maps)

    out = {}
    for g in graph_order:
        m = metas[g]
        n = m["n_total"]
        f1 = np.concatenate([results[k][f"f1_{g}"] for k in range(NCORES)], axis=0)
        f2 = np.concatenate([results[k][f"f2_{g}"] for k in range(NCORES)], axis=0)
        out[g] = (f1[:n].astype(np.float32), f2[:n].astype(np.float32))
    return out


def kernel(
    users_feature, items_feature, bundles_feature,
    ui_vals, bi_vals, ub_vals,
    ui_rows, ui_cols, bi_rows, bi_cols, ub_rows, ub_cols,
):
    users_feature = np.asarray(users_feature, dtype=np.float32)
    items_feature = np.asarray(items_feature, dtype=np.float32)
    bundles_feature = np.asarray(bundles_feature, dtype=np.float32)

    feats = {
        "ui": np.concatenate([users_feature, items_feature], axis=0),
        "bi": np.concatenate([bundles_feature, items_feature], axis=0),
        "ub": np.concatenate([users_feature, bundles_feature], axis=0),
    }
    graphs = {
        "ui": (feats["ui"], ui_rows, ui_cols, ui_vals),
        "bi": (feats["bi"], bi_rows, bi_cols, bi_vals),
        "ub": (feats["ub"], ub_rows, ub_cols, ub_vals),
    }
    graph_order = ["bi", "ub", "ui"]

    fs = _run(graphs, graph_order)

    agg = {}
    for g in graph_order:
        f1, f2 = fs[g]
        agg[g] = (feats[g] + _l2norm_rows(f1) + _l2norm_rows(f2)) / 3.0

    return np.concatenate(
        [
            agg["ui"][:U],
            agg["ub"][:U],
            agg["bi"][:NB],
            agg["ub"][U : U + NB],
            agg["ui"][U : U + NI],
            agg["bi"][NB : NB + NI],
        ],
        axis=0,
    ).astype(np.float32)



# revision 4
# speedup vs baseline: 2.3494x; 1.1273x over previous
"""Trainium2 Bass kernel v2 for multi-graph SpMM propagation.

Computation (per graph): f0 = feat; f_{l+1} = segsum(vals * f_l[cols], rows);
kernel outputs raw f1, f2 shards (fp16). Host applies l2norm + layer average.

v2 vs baseline:
  - tables are host-cast to fp16; every gather is a 256B *pair* gather
    (elem = 2 fp16 rows, idx = col>>1).  Tiles are parity-pure (all even or
    all odd cols), so the matmul rhs is just a 64-col slice of the gathered
    pair buffer -- no extra vector/PE work, no scalar casts.
  - layer 2 gathers straight from the fp16 AllGather output: the fp16->fp32
    expand phase is gone.
  - gathers round-robin over NQUEUES SWDGE queues.
"""

import numpy as np

import concourse.bacc as bacc
import concourse.bass as bass
import concourse.mybir as mybir
import concourse.tile as tile
from concourse.bass_utils import run_bass_kernel_spmd

NCORES = 8
P = 128
D = 64
PAIR_BUCKET = 65536   # pair-index window per gather call (int16, signed)
HALF = 32768
CALL_TILES = 8        # tiles per dma_gather call (1024 idx = SWDGE ring cap)
SEG_BLOCKS = 8        # blocks per segment
NQUEUES = 4

U, NI, NB = 100000, 50000, 20000


def _preprocess_graph(rows, cols, vals, n_total):
    rows = np.asarray(rows, dtype=np.int64)
    cols = np.asarray(cols, dtype=np.int64)
    vals = np.asarray(vals, dtype=np.float32)

    S = ((n_total + NCORES * P - 1) // (NCORES * P)) * P
    nblocks = S // P
    npairs = (n_total + 1) // 2
    nbuckets = max(1, (npairs + PAIR_BUCKET - 1) // PAIR_BUCKET)
    nsegs = (nblocks + SEG_BLOCKS - 1) // SEG_BLOCKS

    core = np.minimum(rows // S, NCORES - 1)
    blk = (rows - core * S) // P
    pair = cols >> 1
    par = (cols & 1).astype(np.int64)
    bkt = pair // PAIR_BUCKET
    seg = blk // SEG_BLOCKS

    # per-bucket gather base (in pair units): mid-window so int16 covers
    # 65536 pairs; clamped near the table end so trailing idx stay >= 0
    bases = []
    for B in range(nbuckets):
        size = npairs - B * PAIR_BUCKET
        if size <= 0:
            bases.append(min(B * PAIR_BUCKET, npairs - 1))
        elif size <= HALF:
            bases.append(B * PAIR_BUCKET)
        elif size <= PAIR_BUCKET:
            bases.append(npairs - HALF)
        else:
            bases.append(B * PAIR_BUCKET + HALF)
    bases = np.array(bases, np.int64)

    # cell id in slot order: (seg, bucket, block, parity)
    ncell_blk = nblocks * 2
    grp = ((seg * nbuckets + bkt) * nblocks + blk) * 2 + par
    ngrp = nsegs * nbuckets * ncell_blk
    counts = np.zeros((NCORES, ngrp), np.int64)
    np.add.at(counts, (core, grp), 1)

    tiles_per_grp = -(-counts.max(axis=0) // P)  # [ngrp]
    # Every block must produce >=1 tile so its PSUM/output gets written.
    tpg = tiles_per_grp.reshape(nsegs, nbuckets, nblocks, 2)
    blk_tiles = tpg.sum(axis=(0, 1, 3))
    for b in np.where(blk_tiles == 0)[0]:
        tpg[b // SEG_BLOCKS, 0, b, 0] = 1
    tiles_per_grp = tpg.reshape(-1)

    grp_tile_start = np.concatenate([[0], np.cumsum(tiles_per_grp)])
    T_tot = int(grp_tile_start[-1])

    # place edges into slots (rank within each (grp, core) run)
    key = grp * NCORES + core
    order = np.argsort(key, kind="stable")
    run_start = np.concatenate([[0], np.cumsum(counts.T.reshape(-1))])
    rank = np.arange(len(rows)) - run_start[key[order]]
    slot = grp_tile_start[grp[order]] * P + rank
    c_s = core[order]

    idx_rel = np.zeros((NCORES, T_tot * P), np.int16)
    rl_slots = np.zeros((NCORES, T_tot * P), np.float32)
    val_slots = np.zeros((NCORES, T_tot * P), np.float32)
    rel = pair[order] - bases[bkt[order]]
    assert rel.min() >= -HALF and rel.max() < HALF
    idx_rel[c_s, slot] = rel.astype(np.int16)
    rl_slots[c_s, slot] = (rows[order] - c_s * S - blk[order] * P).astype(np.float32)
    val_slots[c_s, slot] = vals[order].astype(np.float32)

    # sort each tile's lanes by idx ascending (trailing idx of every gather
    # call must be >= 0: the Q7 kernel trims trailing negatives)
    ir = idx_rel.reshape(NCORES, T_tot, P)
    rl2 = rl_slots.reshape(NCORES, T_tot, P)
    vl2 = val_slots.reshape(NCORES, T_tot, P)
    perm = np.argsort(ir, axis=2, kind="stable")
    ir[:] = np.take_along_axis(ir, perm, axis=2)
    rl2[:] = np.take_along_axis(rl2, perm, axis=2)
    vl2[:] = np.take_along_axis(vl2, perm, axis=2)
    assert (ir[:, :, P - 1] >= 0).all()

    # schedules
    segs = []
    idx_chunks = []  # per-call [NCORES, 16, nt*8] int16
    call_col_off = 0
    idx_t = idx_rel.reshape(NCORES, T_tot, P)
    tile_par = np.zeros(T_tot, np.int8)   # parity of each tile
    tpg4 = tiles_per_grp.reshape(nsegs, nbuckets, nblocks, 2)
    for s in range(nsegs):
        b_lo = s * SEG_BLOCKS
        b_hi = min(nblocks, b_lo + SEG_BLOCKS)
        g_first = ((s * nbuckets + 0) * nblocks + b_lo) * 2
        seg_t0 = int(grp_tile_start[g_first])
        calls = []
        for B in range(nbuckets):
            g0 = ((s * nbuckets + B) * nblocks + b_lo) * 2
            g1 = ((s * nbuckets + B) * nblocks + (b_hi - 1)) * 2 + 1
            t0 = int(grp_tile_start[g0])
            t1 = int(grp_tile_start[g1] + tiles_per_grp[g1])
            t = t0
            while t < t1:
                nt = min(CALL_TILES, t1 - t)
                calls.append(dict(bucket=B, t0=t, nt=nt, col_off=call_col_off))
                chunk = idx_t[:, t : t + nt, :].reshape(NCORES, nt * P)
                wrapped = np.zeros((NCORES, 16, nt * 8), np.int16)
                i = np.arange(nt * P)
                wrapped[:, i % 16, i // 16] = chunk
                idx_chunks.append(wrapped)
                call_col_off += nt * 8
                t += nt
        seg_t1 = t if calls else seg_t0
        blocks = []
        for b in range(b_lo, b_hi):
            tl = []
            for B in range(nbuckets):
                for pq in range(2):
                    g = ((s * nbuckets + B) * nblocks + b) * 2 + pq
                    ts = int(grp_tile_start[g])
                    n = int(tiles_per_grp[g])
                    tl.extend(range(ts, ts + n))
                    tile_par[ts : ts + n] = pq
            blocks.append((b, tl))
        segs.append(dict(seg_t0=seg_t0, seg_t1=seg_t1, calls=calls, blocks=blocks))

    idx_all = np.concatenate(idx_chunks, axis=2)  # [NCORES, 16, cols]
    idx_all = np.tile(idx_all, (1, 8, 1))         # replicate to 128 partitions

    def to_tiles(a):
        return np.ascontiguousarray(a.reshape(NCORES, T_tot, P).transpose(0, 2, 1))

    return dict(
        n_total=n_total, S=S, nblocks=nblocks, nbuckets=nbuckets, bases=bases,
        T_tot=T_tot, segs=segs, idx_ncols=idx_all.shape[2], tile_par=tile_par,
        idx=np.ascontiguousarray(idx_all),
        rl=to_tiles(rl_slots), val=to_tiles(val_slots),
    )


def _build_program(metas, graph_order, repeat=1, knockout=()):
    """knockout (timing diagnostics only): any of {"gather","onehot","matmul",
    "coll","out"} — skips emitting that component (results become garbage)."""
    f16 = mybir.dt.float16
    f32 = mybir.dt.float32
    i16 = mybir.dt.int16

    nc = bacc.Bacc(
        "TRN2", target_bir_lowering=False, debug=False,
        enable_asserts=False, num_devices=NCORES,
        num_swdge_queues=NQUEUES,
    )

    tabs, idxs, rls, vls, f1s, f2s = {}, {}, {}, {}, {}, {}
    for g in graph_order:
        m = metas[g]
        # fp16 table, padded to even row count for pair gathers
        nrows16 = 2 * ((m["n_total"] + 1) // 2)
        tabs[g] = nc.dram_tensor(f"tab16_{g}", [nrows16, D], f16, kind="ExternalInput")
        idxs[g] = nc.dram_tensor(f"idx_{g}", [P, m["idx_ncols"]], i16, kind="ExternalInput")
        rls[g] = nc.dram_tensor(f"rl_{g}", [P, m["T_tot"]], f32, kind="ExternalInput")
        vls[g] = nc.dram_tensor(f"val_{g}", [P, m["T_tot"]], f32, kind="ExternalInput")
        f1s[g] = nc.dram_tensor(f"f1_{g}", [m["S"], D], f16, kind="ExternalOutput")
        f2s[g] = nc.dram_tensor(f"f2_{g}", [m["S"], D], f16, kind="ExternalOutput")

    seg_tiles_max = max(
        max(s["seg_t1"] - s["seg_t0"] for s in metas[g]["segs"]) for g in graph_order
    )
    seg_cols_max = max(
        max(max(sum(c["nt"] for c in s["calls"]) * 8, 1) for s in metas[g]["segs"])
        for g in graph_order
    )

    with tile.TileContext(nc) as tc:
        with (
            tc.tile_pool(name="const", bufs=1) as cpool,
            tc.tile_pool(name="meta", bufs=1) as mpool,
            tc.tile_pool(name="idxp", bufs=3) as ipool,
            tc.tile_pool(name="segp", bufs=3) as spool,
            tc.tile_pool(name="oneh", bufs=16) as opool,
            tc.tile_pool(name="fout", bufs=8) as fpool,
            tc.tile_pool(name="psum", bufs=6, space="PSUM") as ppool,
            tc.tile_pool(name="dram", bufs=1, space="DRAM") as dpool,
        ):
            iota_t = cpool.tile([P, P], i16, name="iota_t")
            nc.gpsimd.iota(iota_t[:], pattern=[[1, P]], base=0, channel_multiplier=0)

            rl_sb, val_sb, ag_in, table2 = {}, {}, {}, {}
            for g in graph_order:
                m = metas[g]
                rl_sb[g] = mpool.tile([P, m["T_tot"]], f32, tag=f"rl_{g}", name=f"rl_sb_{g}")
                val_sb[g] = mpool.tile([P, m["T_tot"]], f32, tag=f"val_{g}", name=f"val_sb_{g}")
                nc.sync.dma_start(rl_sb[g][:], rls[g][:])
                nc.sync.dma_start(val_sb[g][:], vls[g][:])
                ag_in[g] = dpool.tile([m["S"], D], f16, tag=f"agin_{g}", name=f"ag_in_{g}")
                # pad AllGather'd table to even rows (S*NCORES is even)
                table2[g] = dpool.tile([m["S"] * NCORES, D], f16, tag=f"tab2_{g}", name=f"table2_{g}")

            qctr = [0]

            def emit_layer(g, table_ap_fn, dst_drams):
                m = metas[g]
                tile_par = m["tile_par"]
                for s in m["segs"]:
                    seg_t0 = s["seg_t0"]
                    segbuf = None
                    if s["calls"]:
                        ncols = sum(c["nt"] for c in s["calls"]) * 8
                        idxchunk = ipool.tile([P, seg_cols_max], i16, tag="idxc", name="idxchunk")
                        c0 = s["calls"][0]["col_off"]
                        nc.sync.dma_start(idxchunk[:, :ncols], idxs[g][:, c0 : c0 + ncols])
                        segbuf = spool.tile([P, seg_tiles_max, 2 * D], f16, tag="segbuf", name="segbuf")
                        for call in s["calls"]:
                            nt = call["nt"]
                            so = call["t0"] - seg_t0
                            if "gather" in knockout:
                                continue
                            nc.gpsimd.dma_gather(
                                out_ap=segbuf[:, so : so + nt, :],
                                in_ap=table_ap_fn(call["bucket"]),
                                idxs_ap=idxchunk[:, call["col_off"] - c0 : call["col_off"] - c0 + nt * 8],
                                num_idxs=nt * P,
                                num_idxs_reg=nt * P,
                                elem_size=2 * D,
                                queue_num=qctr[0] % NQUEUES,
                            )
                            qctr[0] += 1
                    for b, tl in s["blocks"]:
                        psum = ppool.tile([P, D], f32, tag="psum", name="psum")
                        for i, t in enumerate(tl):
                            onehot = opool.tile([P, P], f16, tag="oneh", name="onehot")
                            if "onehot" not in knockout:
                                nc.vector.tensor_scalar(
                                    onehot[:], iota_t[:],
                                    rl_sb[g][:, t : t + 1], val_sb[g][:, t : t + 1],
                                    mybir.AluOpType.is_equal, mybir.AluOpType.mult,
                                )
                            pq = int(tile_par[t])
                            if "matmul" not in knockout:
                                nc.tensor.matmul(
                                    out=psum[:], lhsT=onehot[:],
                                    rhs=segbuf[:, t - seg_t0, pq * D : (pq + 1) * D],
                                    start=(i == 0), stop=(i == len(tl) - 1),
                                )
                        if "matmul" in knockout or "out" in knockout:
                            continue
                        f16t = fpool.tile([P, D], f16, tag="fout", name="f16t")
                        nc.scalar.activation(f16t[:], psum[:], mybir.ActivationFunctionType.Copy)
                        for dd in dst_drams:
                            nc.sync.dma_start(dd[b * P : (b + 1) * P, :], f16t[:])

            rg = [list(range(NCORES))]
            for _rep in range(repeat):
                for g in graph_order:
                    emit_layer(
                        g,
                        lambda B, g=g: tabs[g][2 * int(metas[g]["bases"][B]) :, :].rearrange(
                            "(a b) d -> a (b d)", b=2),
                        [ag_in[g], f1s[g]],
                    )
                    if "coll" in knockout:
                        continue
                    nc.gpsimd.collective_compute(
                        "AllGather", mybir.AluOpType.bypass, replica_groups=rg,
                        ins=[ag_in[g][:]], outs=[table2[g][:]],
                    )
                for g in graph_order:
                    emit_layer(
                        g,
                        lambda B, g=g: table2[g][2 * int(metas[g]["bases"][B]) :, :].rearrange(
                            "(a b) d -> a (b d)", b=2),
                        [f2s[g]],
                    )

    nc.compile()
    return nc


def _l2norm_rows(x):
    x = x.astype(np.float32)
    n = np.sqrt(np.sum(x * x, axis=1, keepdims=True))
    return x / np.maximum(n, 1e-12)


def _make_in_maps(graphs, graph_order, metas):
    in_maps = []
    for k in range(NCORES):
        im = {}
        for g in graph_order:
            feat = graphs[g][0]
            m = metas[g]
            nrows16 = 2 * ((m["n_total"] + 1) // 2)
            t16 = np.zeros((nrows16, D), np.float16)
            t16[: m["n_total"]] = feat.astype(np.float16)
            im[f"tab16_{g}"] = t16
            im[f"idx_{g}"] = np.ascontiguousarray(m["idx"][k])
            im[f"rl_{g}"] = np.ascontiguousarray(m["rl"][k])
            im[f"val_{g}"] = np.ascontiguousarray(m["val"][k])
        in_maps.append(im)
    return in_maps


def _run(graphs, graph_order, run_fn=None):
    metas = {g: _preprocess_graph(graphs[g][1], graphs[g][2], graphs[g][3],
                                  graphs[g][0].shape[0]) for g in graph_order}
    nc = _build_program(metas, graph_order)
    in_maps = _make_in_maps(graphs, graph_order, metas)

    if run_fn is None:
        results = run_bass_kernel_spmd(nc, in_maps, core_ids=list(range(NCORES))).results
    else:
        results = run_fn(nc, in_maps)

    out = {}
    for g in graph_order:
        m = metas[g]
        n = m["n_total"]
        f1 = np.concatenate([results[k][f"f1_{g}"] for k in range(NCORES)], axis=0)
        f2 = np.concatenate([results[k][f"f2_{g}"] for k in range(NCORES)], axis=0)
        out[g] = (f1[:n].astype(np.float32), f2[:n].astype(np.float32))
    return out


def kernel(
    users_feature, items_feature, bundles_feature,
    ui_vals, bi_vals, ub_vals,
    ui_rows, ui_cols, bi_rows, bi_cols, ub_rows, ub_cols,
):
    users_feature = np.asarray(users_feature, dtype=np.float32)
    items_feature = np.asarray(items_feature, dtype=np.float32)
    bundles_feature = np.asarray(bundles_feature, dtype=np.float32)

    feats = {
        "ui": np.concatenate([users_feature, items_feature], axis=0),
        "bi": np.concatenate([bundles_feature, items_feature], axis=0),
        "ub": np.concatenate([users_feature, bundles_feature], axis=0),
    }
    graphs = {
        "ui": (feats["ui"], ui_rows, ui_cols, ui_vals),
        "bi": (feats["bi"], bi_rows, bi_cols, bi_vals),
        "ub": (feats["ub"], ub_rows, ub_cols, ub_vals),
    }
    graph_order = ["bi", "ub", "ui"]

    fs = _run(graphs, graph_order)

    agg = {}
    for g in graph_order:
        f1, f2 = fs[g]
        agg[g] = (feats[g] + _l2norm_rows(f1) + _l2norm_rows(f2)) / 3.0

    return np.concatenate(
        [
            agg["ui"][:U],
            agg["ub"][:U],
            agg["bi"][:NB],
            agg["ub"][U : U + NB],
            agg["ui"][U : U + NI],
            agg["bi"][NB : NB + NI],
        ],
        axis=0,
    ).astype(np.float32)
